# revision 53
# baseline (speedup 1.0000x reference)
"""TRN2 Bass kernel for nn_Attention_m_17815524344494.

Multi-head attention over [B=8, M=4, P=512, H=768], nh=12, hs=64.
Sharding: data-parallel over batch B -> one batch element per NeuronCore (8 cores).

Per-core dataflow (T = M*P = 2048 tokens; all matmul operands fp16 —
same 11-bit mantissa as float32r but FWL-eligible so LDWEIGHTS hides;
accumulation is always fp32 in PSUM):
  1. xT [768,2048] (pre-transposed on host) DMA'd feature-major per modality
  2. qT = Wq^T xT, kT = Wk^T xT (feature-major), v = x Wv (token-major,
     augmented with a ones column per head for free softmax sums)
  3. per (modality, head): scoresT = kT^T q (keys on partitions),
     eT = exp(scoresT/8) via ScalarE, ctxT_unnorm/sums = v_aug^T eT,
     1/sums via reciprocal_approx_fast, partition-broadcast through a
     DRAM bounce DMA, normalize in place on VectorE
  4. out = ctxT^T Wo (token-major), DMA to DRAM

Biases are zeros per the problem spec; a numpy fallback handles the
(never exercised) nonzero-bias case.
"""

from contextlib import ExitStack

import numpy as np

import concourse.mybir as mybir
from concourse import bacc, bass_utils
from concourse.tile import TileContext

F32 = mybir.dt.float32
F32R = mybir.dt.float32r
F16 = mybir.dt.float16
AF = mybir.ActivationFunctionType
ALU = mybir.AluOpType

B, M, PM, H = 8, 4, 512, 768
NH, HS = 12, 64
T = M * PM          # 2048 tokens per core
HC = H // 128       # 6 hidden chunks
TCM = PM // 128     # 4 token chunks per modality


def _emit(tc, ctx):
    nc = tc.nc

    x_ap = nc.dram_tensor("x", [H, T], F16, kind="ExternalInput").ap()
    wq_ap = nc.dram_tensor("wq", [H, H], F16, kind="ExternalInput").ap()
    wk_ap = nc.dram_tensor("wk", [H, H], F16, kind="ExternalInput").ap()
    wv_ap = nc.dram_tensor("wv", [H, H], F16, kind="ExternalInput").ap()
    wo_ap = nc.dram_tensor("wo", [H, H], F16, kind="ExternalInput").ap()
    out_ap = nc.dram_tensor("out", [T, H], F16, kind="ExternalOutput").ap()
    srf_ap = nc.dram_tensor("srf", [M * NH, 512], F32, kind="Internal").ap()

    const = ctx.enter_context(tc.tile_pool(name="const", bufs=1))

    # f32r tiles can't be written by memset/affine_select directly (no
    # f32r rounding on those ISA paths); stage in f32 and copy via DVE.
    onescol = const.tile([128, NH * TCM], F16)
    onesrow = const.tile([1, 128], F16)
    warm = const.tile([128, 512], F16)
    nc.gpsimd.memset(warm[:], 0.0)
    expwarm = const.tile([1, 16], F32)
    with tc.tile_pool(name="stage", bufs=1) as stage:
        ones_stage = stage.tile([128, 128], F32)
        nc.gpsimd.memset(ones_stage[:], 1.0)
        nc.vector.tensor_copy(onescol[:], ones_stage[:, :NH * TCM])
        nc.vector.tensor_copy(onesrow[:], ones_stage[0:1, :])

    wpool = ctx.enter_context(tc.tile_pool(name="w", bufs=1))
    xtp = ctx.enter_context(tc.tile_pool(name="xt", bufs=2))
    qpool = ctx.enter_context(tc.tile_pool(name="q", bufs=2))
    kpool = ctx.enter_context(tc.tile_pool(name="k", bufs=2))
    vpool = ctx.enter_context(tc.tile_pool(name="v", bufs=2))
    epool = ctx.enter_context(tc.tile_pool(name="e", bufs=4))
    stpool = ctx.enter_context(tc.tile_pool(name="st", bufs=4))
    bcpool = ctx.enter_context(tc.tile_pool(name="bc", bufs=5))
    cpool = ctx.enter_context(tc.tile_pool(name="ctx", bufs=1))
    opool = ctx.enter_context(tc.tile_pool(name="o", bufs=4))
    ps_big = ctx.enter_context(tc.tile_pool(name="ps_big", bufs=2, space="PSUM"))
    ps_sc = ctx.enter_context(tc.tile_pool(name="ps_sc", bufs=2, space="PSUM"))
    ps_c = ctx.enter_context(tc.tile_pool(name="ps_c", bufs=2, space="PSUM"))

    w_tiles = {}

    def load_weights():
        # wk behind x0 on the scalar queue; wv/wo behind wq on sync, so the
        # k-projection bootstrap isn't starved waiting for wv/wo bytes.
        for name, ap, eng in (("wk", wk_ap, nc.sync), ("wv", wv_ap, nc.scalar),
                              ("wo", wo_ap, nc.scalar)):
            t = wpool.tile([128, HC, H], F16, tag=name)
            src = ap.rearrange("(kc p) j -> p kc j", p=128)
            for kc in range(HC):
                eng.dma_start(t[:, kc, :], src[:, kc, :])
            w_tiles[name] = t

    mod = {}

    def emit_load_x(m):
        xt = xtp.tile([128, HC, PM], F16, tag="xt")
        if m == 0:
            # Interleave x and Wq chunk DMAs on the two HW DGE queues so the
            # first projection group's operands land as early as possible.
            wq = wpool.tile([128, HC, H], F16, tag="wq", name="wq")
            w_tiles["wq"] = wq
            wq_src = wq_ap.rearrange("(kc p) j -> p kc j", p=128)
            for hc in range(HC):
                nc.scalar.dma_start(
                    xt[:, hc, :],
                    x_ap.rearrange("(hc p) t -> p hc t", p=128)[:, hc, :PM],
                )
                nc.sync.dma_start(wq[:, hc, :], wq_src[:, hc, :])
            mod[m] = {"xt": xt}
            load_weights()
            return
        for hc in range(HC):
            nc.scalar.dma_start(
                xt[:, hc, :],
                x_ap.rearrange("(hc p) t -> p hc t", p=128)[:, hc, m * PM:(m + 1) * PM],
            )
        mod[m] = {"xt": xt}

    def evac_k(kt, jc, ps, eng_idx):
        # kt is the zero-padded per-head slab layout [128, NH, PM]: head 2jc
        # lives in rows 0-63 of slab 2jc, head 2jc+1 in rows 64-127 of slab
        # 2jc+1; the complementary halves stay zero (boot memset) so scores
        # can contract over all 128 partitions.
        for half, hh in ((0, 2 * jc), (64, 2 * jc + 1)):
            src = ps[half:half + 64, :]
            dst = kt[half:half + 64, hh, :]
            if eng_idx % 2 == 0:
                nc.vector.tensor_copy(dst, src)
            else:
                nc.scalar.activation(dst, src, AF.Copy)

    def proj_qk_group(m, which, jc):
        st = mod[m]
        key = "qt" if which == "q" else "kt"
        if key not in st:
            if which == "q":
                st[key] = qpool.tile([128, HC, PM], F16, tag="q", name="qt")
            else:
                st[key] = kpool.tile([128, NH, PM], F16, tag="k", name="kt")
        w = w_tiles["wq" if which == "q" else "wk"]
        ps = ps_big.tile([128, 512], F32, tag="ps_big")
        for kc in range(HC):
            nc.tensor.matmul(
                ps[:],
                w[:, kc, jc * 128:(jc + 1) * 128],
                st["xt"][:, kc, :],
                start=(kc == 0),
                stop=(kc == HC - 1),
            )
        if which == "k":
            evac_k(st[key], jc, ps, jc)
        elif jc % 2 == 0:
            nc.vector.tensor_copy(st[key][:, jc, :], ps[:])
        else:
            nc.scalar.activation(st[key][:, jc, :], ps[:], AF.Copy)

    def proj_v_group(m, ti, nn):
        # vt is flat [128, TCM, NH*(HS+1)+63]: head h's (v | ones) slab at
        # offset h*65; the +63 tail lets the PV stationary read a full
        # 128-column slab even for the last head (overreads are finite
        # garbage that lands in unread PSUM rows 65..127).
        st = mod[m]
        if "vt" not in st:
            st["vt"] = vpool.tile([128, TCM, NH * (HS + 1) + 63], F16,
                                  tag="v", name="vt")
            nc.vector.tensor_copy(
                st["vt"][:, :, :NH * (HS + 1)]
                .rearrange("p t (h c) -> p t h c", c=HS + 1)[:, :, :, HS],
                onescol[:].rearrange("p (t h) -> p t h", t=TCM),
            )
        ps = ps_big.tile([128, 512], F32, tag="ps_big")
        for kc in range(HC):
            nc.tensor.matmul(
                ps[:, :384],
                st["xt"][:, kc, ti * 128:(ti + 1) * 128],
                w_tiles["wv"][:, kc, nn * 384:(nn + 1) * 384],
                start=(kc == 0),
                stop=(kc == HC - 1),
            )
        nc.scalar.activation(
            st["vt"][:, ti, nn * 390:(nn + 1) * 390]
            .rearrange("p (h c) -> p h c", c=HS + 1)[:, :, :HS],
            ps[:, :384].rearrange("p (h c) -> p h c", c=HS),
            AF.Copy,
        )

    def phase_ab_fillers(m):
        # qk groups are the per-head main fillers: they share the attention
        # matmuls' 128*128*512 config, so weaving them costs no PE reconfig.
        # The 384-wide v groups are spread as second fillers over the later
        # heads (a block at the boundary stalls on burst Scalar evacs).
        main = [lambda: emit_load_x(m)]
        for which in ("q", "k"):
            for jc in range(HC):
                main.append(lambda which=which, jc=jc: proj_qk_group(m, which, jc))
        vlist = [lambda ti=ti, nn=nn: proj_v_group(m, ti, nn)
                 for ti in range(TCM) for nn in range(2)]
        return main, vlist

    def drain_bcs(ctxt, bcs):
        for hh, bcv in bcs:
            hhc, hhr = hh // 2, (hh % 2) * 64
            nc.vector.tensor_tensor(
                ctxt[hhr:hhr + 64, hhc, :], ctxt[hhr:hhr + 64, hhc, :],
                bcv[hhr:hhr + 64, :], ALU.mult,
            )
        del bcs[:]

    def finish_head(m, item):
        # Fused evacuate+normalize: ctxt[h] = psc[:64] * (1/sums broadcast).
        # For the bounce path the broadcast arrived by DMA; for the PE path
        # broadcast 1/sums to all 128 partitions with a contract-1 matmul
        # (fills PE bubbles in the fillerless last modality).
        h, psc, bcv, rf16 = item
        hc, hr = h // 2, (h % 2) * 64
        if rf16 is not None:
            psb = ps_sc.tile([128, 512], F32, tag="ps_sc", name="psbc")
            nc.tensor.matmul(psb[:], onesrow[0:1, :], rf16[0:1, :],
                             start=True, stop=True)
            bcv = bcpool.tile([128, 512], F32, tag="bc", name="bcsb")
            nc.vector.tensor_copy(bcv[hr:hr + 64, :], psb[hr:hr + 64, :])
        nc.vector.tensor_tensor(
            mod[m]["ctxt"][hr:hr + 64, hc, :], psc[:HS, :],
            bcv[hr:hr + 64, :], ALU.mult,
        )

    def attention(m, main, vlist, bcast=False):
        # Per (modality, head): scoresT on PE, exp on ScalarE, PV (with the
        # v_aug ones column producing softmax sums in psum row 64), then
        # 1/sums via reciprocal_approx_fast straight from PSUM and a fused
        # evacuate+normalize on DVE one head later.  Between each head's
        # scores and PV we weave one independent PE work unit (projections of
        # the next modality, or out-proj of the previous one) to fill the
        # exp wait.
        st = mod[m]
        qt, kt, vt = st["qt"], st["kt"], st["vt"]
        ctxt = cpool.tile([128, HC, PM], F16, tag="ctx")
        st["ctxt"] = ctxt
        pend = []
        bcs = []

        for h in range(NH):
            hc, hr = h // 2, (h % 2) * 64
            # kt slabs are zero-padded to 128 contract rows and vt slabs to
            # 128 stationary columns, so every attention matmul is the same
            # 128*128*512 config as the projections -- no PE reconfig
            # penalty between scores / PV / filler groups.
            qh = qt[:, hc, :]
            ets = []
            for jc2 in range(2):
                pst = ps_sc.tile([128, 2, 512], F32, tag="ps_sc",
                                 name=f"pst{jc2}")
                for j in range(2):
                    jc = jc2 * 2 + j
                    nc.tensor.matmul(
                        pst[:, j, :],
                        kt[:, h, jc * 128:(jc + 1) * 128],
                        qh,
                        start=True,
                        stop=True,
                    )
                et = epool.tile([128, 1024], F16, tag="e")
                nc.scalar.activation(et[:], pst[:], AF.Exp, scale=0.125)
                ets.append(et)
            if main:
                main.pop(0)()
                if h == 0 and len(main) > NH:
                    main.pop(0)()
            elif vlist:
                vlist.pop(0)()
            psc = ps_c.tile([128, 512], F32, tag="ps_c")
            for jc in range(TCM):
                nc.tensor.matmul(
                    psc[:],
                    vt[:, jc, h * (HS + 1):h * (HS + 1) + 128],
                    ets[jc // 2][:, (jc % 2) * 512:(jc % 2 + 1) * 512],
                    start=(jc == 0),
                    stop=(jc == TCM - 1),
                )
            stmp = stpool.tile([1, 512], F32, tag="stmp")
            nc.vector.tensor_copy(stmp[:], psc[HS:HS + 1, :])
            rf = stpool.tile([1, 512], F32, tag="rf")
            nc.vector.reciprocal_approx_fast(out=rf[:], in_=stmp[:])
            if not bcast:
                # Decoupled: plain evac releases the PSUM bank immediately;
                # the in-place normalize runs later, once the DMA-bounce
                # broadcast lands (batched so the PE never waits on it).
                nc.vector.tensor_copy(ctxt[hr:hr + 64, hc, :], psc[:HS, :])
                row = srf_ap[m * NH + h:m * NH + h + 1, :]
                nc.sync.dma_start(row, rf[0:1, :])
                bc = bcpool.tile([128, 512], F32, tag="bc")
                nc.sync.dma_start(bc[hr:hr + 64, :], row.to_broadcast((64, 512)))
                bcs.append((h, bc))
                if len(bcs) == 4 and h < NH - 1:
                    drain_bcs(ctxt, bcs)
            else:
                rf16 = stpool.tile([1, 512], F16, tag="rf16")
                nc.scalar.activation(rf16[:], rf[:], AF.Copy)
                pend.append((h, psc, None, rf16))
                while len(pend) > 1:
                    finish_head(m, pend.pop(0))
            if h >= NH - 8 and vlist:
                vlist.pop(0)()
        for f in main + vlist:
            f()
        del main[:], vlist[:]
        drain_bcs(ctxt, bcs)
        return pend

    def out_proj(m, pend=None, split=False):
        # For the final modality the last heads' normalize is still in
        # flight: emit the first two token tiles' cc0..4 partial sums to
        # cover it, then close with cc5 + evacuations.
        ctxt = mod[m]["ctxt"]

        def emit_mms(ti, nn, ps, ccs):
            for cc in ccs:
                nc.tensor.matmul(
                    ps[:, :384],
                    ctxt[:, cc, ti * 128:(ti + 1) * 128],
                    w_tiles["wo"][:, cc, nn * 384:(nn + 1) * 384],
                    start=(cc == 0),
                    stop=(cc == HC - 1),
                )

        def emit_tail(ti, nn, ps, osb):
            emit_mms(ti, nn, ps, range(5, HC))
            nc.scalar.activation(osb[:, nn * 384:(nn + 1) * 384],
                                 ps[:, :384], AF.Copy)

        if split:
            # cc0..3 only touches heads 0-7 (normalized long ago); emit all
            # four token tiles' partial sums first so the PE keeps running
            # while the last heads' normalize (bc bounce) lands, then close
            # each tile with cc4..5 + evacuation.
            psE = ps_sc.tile([128, 2, 512], F32, tag="ps_sc", name="psoE")
            psF = ps_sc.tile([128, 2, 512], F32, tag="ps_sc", name="psoF")
            psts = [
                ps_big.tile([128, 512], F32, tag="ps_big", name="psoA")[:],
                ps_big.tile([128, 512], F32, tag="ps_big", name="psoB")[:],
                psE[:, 0, :], psE[:, 1, :],
                psF[:, 0, :], psF[:, 1, :],
                ps_c.tile([128, 512], F32, tag="ps_c", name="psoG")[:],
                ps_c.tile([128, 512], F32, tag="ps_c", name="psoH")[:],
            ]
            for ti in range(TCM):
                for nn in range(2):
                    emit_mms(ti, nn, psts[ti * 2 + nn], range(4))
            while pend:
                finish_head(m, pend.pop(0))

            # Close cc4..5 one tile ahead of the evacuations, with the two
            # half-evacs split across Scalar and Vector, so the final drain
            # overlaps instead of serializing on one engine.
            osbs = [opool.tile([128, H], F16, tag="o", name=f"osbf{ti}")
                    for ti in range(TCM)]

            def close_mms(ti):
                for nn in range(2):
                    emit_mms(ti, nn, psts[ti * 2 + nn], range(4, HC))

            def close_evac(ti):
                nc.scalar.activation(osbs[ti][:, :384],
                                     psts[ti * 2][:, :384], AF.Copy)
                nc.vector.tensor_copy(osbs[ti][:, 384:],
                                      psts[ti * 2 + 1][:, :384])
                row0 = (m * TCM + ti) * 128
                nc.sync.dma_start(out_ap[row0:row0 + 128, :], osbs[ti][:])

            close_mms(0)
            close_mms(1)
            close_evac(0)
            close_mms(2)
            close_evac(1)
            close_mms(3)
            close_evac(2)
            close_evac(3)
            return
        for ti in range(TCM):
            osb = opool.tile([128, H], F16, tag="o")
            for nn in range(2):
                ps = ps_big.tile([128, 512], F32, tag="ps_big")
                emit_mms(ti, nn, ps, range(HC))
                nc.scalar.activation(osb[:, nn * 384:(nn + 1) * 384],
                                     ps[:, :384], AF.Copy)
            row0 = (m * TCM + ti) * 128
            nc.sync.dma_start(out_ap[row0:row0 + 128, :], osb[:])

    # PE warmup: dependency-free matmuls on a zeroed tile kick the DVFS
    # ramp while the first x/W DMA chunks are still in flight; also preload
    # the EXP activation table so the first real exp doesn't pay the
    # ~1.5us table load on the critical path.  The kt pad-halves are zeroed
    # once here (DVE is idle); the k evacs only ever write the live halves,
    # so the zeros survive the kpool buffer rotation across modalities.
    nc.scalar.activation(expwarm[:], warm[0:1, :16], AF.Exp, scale=0.125)
    for i in range(2):
        ktz = kpool.tile([128, NH, PM], F16, tag="k", name=f"ktz{i}")
        nc.vector.memset(ktz[0:64, 1::2, :], 0.0)
        nc.vector.memset(ktz[64:128, 0::2, :], 0.0)
    for _ in range(10):
        ps = ps_big.tile([128, 512], F32, tag="ps_big")
        nc.tensor.matmul(ps[:], warm[:, :128], warm[:], start=True, stop=True)

    # Modality 0 bootstrap: kc-outer paired projection consumes x/W DMA
    # chunks as they arrive instead of waiting for whole tensors.
    emit_load_x(0)
    st0 = mod[0]
    st0["qt"] = qpool.tile([128, HC, PM], F16, tag="q", name="qt0")
    st0["kt"] = kpool.tile([128, NH, PM], F16, tag="k", name="kt0")
    for which in ("q", "k"):
        key = "qt" if which == "q" else "kt"
        w = w_tiles["wq" if which == "q" else "wk"]
        for jcp in range(3):
            psA = ps_big.tile([128, 512], F32, tag="ps_big")
            psB = ps_sc.tile([128, 512], F32, tag="ps_sc")
            for kc in range(HC):
                nc.tensor.matmul(
                    psA[:], w[:, kc, (2 * jcp) * 128:(2 * jcp + 1) * 128],
                    st0["xt"][:, kc, :], start=(kc == 0), stop=(kc == HC - 1))
                nc.tensor.matmul(
                    psB[:], w[:, kc, (2 * jcp + 1) * 128:(2 * jcp + 2) * 128],
                    st0["xt"][:, kc, :], start=(kc == 0), stop=(kc == HC - 1))
            if which == "k":
                evac_k(st0[key], 2 * jcp, psA, 0)
                evac_k(st0[key], 2 * jcp + 1, psB, 0)
            else:
                nc.vector.tensor_copy(st0[key][:, 2 * jcp, :], psA[:])
                nc.vector.tensor_copy(st0[key][:, 2 * jcp + 1, :], psB[:])
    for ti in range(TCM):
        for nn in range(2):
            proj_v_group(0, ti, nn)
    def op2_filler_units(m):
        # out_proj(m) sliced into per-(ti, nn) units, used as PE fillers
        # inside the (otherwise fillerless) last modality's attention.
        state = {}

        def unit(ti, nn):
            ctxt = mod[m]["ctxt"]
            if ti not in state:
                state[ti] = opool.tile([128, H], F16, tag="o", name=f"osb2_{ti}")
            osb = state[ti]
            ps = ps_big.tile([128, 512], F32, tag="ps_big")
            for cc in range(HC):
                nc.tensor.matmul(
                    ps[:, :384],
                    ctxt[:, cc, ti * 128:(ti + 1) * 128],
                    w_tiles["wo"][:, cc, nn * 384:(nn + 1) * 384],
                    start=(cc == 0),
                    stop=(cc == HC - 1),
                )
            nc.scalar.activation(osb[:, nn * 384:(nn + 1) * 384],
                                 ps[:, :384], AF.Copy)
            if nn == 1:
                row0 = (m * TCM + ti) * 128
                nc.sync.dma_start(out_ap[row0:row0 + 128, :], osb[:])

        return [lambda ti=ti, nn=nn: unit(ti, nn)
                for ti in range(TCM) for nn in range(2)]

    for m in range(M):
        if m + 1 < M:
            main, vlist = phase_ab_fillers(m + 1)
        else:
            main, vlist = op2_filler_units(M - 2), []
        pend = attention(m, main, vlist, bcast=False)
        if m == M - 1:
            out_proj(m, pend, split=True)
        else:
            while pend:
                finish_head(m, pend.pop(0))
            if m != M - 2:
                out_proj(m)

_NC_CACHE = {}


def build_nc():
    if "nc" not in _NC_CACHE:
        nc = bacc.Bacc("TRN2", target_bir_lowering=False, debug=False, num_devices=B)
        with TileContext(nc) as tc:
            with ExitStack() as stack:
                _emit(tc, stack)
        nc.compile()
        _NC_CACHE["nc"] = nc
    return _NC_CACHE["nc"]


def _numpy_fallback(x, Wq, bq, Wk, bk, Wv, bv, Wo, bo):
    Bb, Mm, Pp, Hh = x.shape
    xx = x.reshape(-1, Hh)
    q = (xx @ Wq + bq).reshape(Bb, Mm, Pp, NH, HS).transpose(0, 1, 3, 2, 4)
    k = (xx @ Wk + bk).reshape(Bb, Mm, Pp, NH, HS).transpose(0, 1, 3, 2, 4)
    v = (xx @ Wv + bv).reshape(Bb, Mm, Pp, NH, HS).transpose(0, 1, 3, 2, 4)
    s = np.einsum("bmnqh,bmnkh->bmnqk", q, k) / np.sqrt(HS)
    s = s - s.max(axis=-1, keepdims=True)
    e = np.exp(s)
    p = e / e.sum(axis=-1, keepdims=True)
    ctx = np.einsum("bmnqk,bmnkh->bmnqh", p, v)
    ctx = ctx.transpose(0, 1, 3, 2, 4).reshape(Bb, Mm, Pp, Hh)
    return (ctx @ Wo + bo).astype(np.float32)


def make_in_maps(hs, ws):
    """hs: [B,M,P,H] f32, ws: dict of f32 [H,H] -> per-core fp16 input maps."""
    ws16 = {n: np.ascontiguousarray(w.astype(np.float16)) for n, w in ws.items()}
    return [
        {"x": np.ascontiguousarray(hs[b].reshape(T, H).T.astype(np.float16)),
         **ws16}
        for b in range(B)
    ]


def kernel(hidden_states, Wq, bq, Wk, bk, Wv, bv, Wo, bo):
    hs = np.ascontiguousarray(np.asarray(hidden_states, dtype=np.float32))
    ws = {n: np.ascontiguousarray(np.asarray(w, dtype=np.float32))
          for n, w in (("wq", Wq), ("wk", Wk), ("wv", Wv), ("wo", Wo))}
    biases = [np.asarray(b, dtype=np.float32) for b in (bq, bk, bv, bo)]
    if any(np.any(b) for b in biases):
        return _numpy_fallback(hs, ws["wq"], biases[0], ws["wk"], biases[1],
                               ws["wv"], biases[2], ws["wo"], biases[3])

    in_maps = make_in_maps(hs, ws)
    # The device occasionally comes up wedged from a previous process
    # (NRT_EXEC_UNIT_UNRECOVERABLE); retry, then degrade to the (correct
    # but slow) numpy path rather than crash.
    last_exc = None
    for _ in range(3):
        try:
            nc = build_nc()
            res = bass_utils.run_bass_kernel_spmd(
                nc, in_maps, core_ids=list(range(B)))
            out = np.stack(
                [res.results[b]["out"].reshape(M, PM, H) for b in range(B)])
            return out.astype(np.float32)
        except Exception as e:  # noqa: BLE001
            last_exc = e
            import time
            time.sleep(2)
    import warnings
    warnings.warn(f"TRN execution failed ({last_exc!r}); numpy fallback")
    return _numpy_fallback(hs, ws["wq"], biases[0], ws["wk"], biases[1],
                           ws["wv"], biases[2], ws["wo"], biases[3])



# revision 54
# speedup vs baseline: 1.1544x; 1.1544x over previous
"""TRN2 Bass kernel for nn_Attention_m_17815524344494.

Multi-head attention over [B=8, M=4, P=512, H=768], nh=12, hs=64.
Sharding: data-parallel over batch B -> one batch element per NeuronCore (8 cores).

Per-core dataflow (T = M*P = 2048 tokens; all matmul operands fp16 —
same 11-bit mantissa as float32r but FWL-eligible so LDWEIGHTS hides;
accumulation is always fp32 in PSUM):
  1. xT [768,2048] (pre-transposed on host) DMA'd feature-major per modality
  2. qT = Wq^T xT, kT = Wk^T xT (feature-major), v = x Wv (token-major,
     augmented with a ones column per head for free softmax sums)
  3. per (modality, head): scoresT = kT^T q (keys on partitions),
     eT = exp(scoresT/8) via ScalarE, ctxT_unnorm/sums = v_aug^T eT,
     1/sums via reciprocal_approx_fast, partition-broadcast through a
     DRAM bounce DMA, normalize in place on VectorE
  4. out = ctxT^T Wo (token-major), DMA to DRAM

Biases are zeros per the problem spec; a numpy fallback handles the
(never exercised) nonzero-bias case.
"""

from contextlib import ExitStack

import numpy as np

import concourse.mybir as mybir
from concourse import bacc, bass_utils
from concourse.tile import TileContext

F32 = mybir.dt.float32
F32R = mybir.dt.float32r
F16 = mybir.dt.float16
AF = mybir.ActivationFunctionType
ALU = mybir.AluOpType

B, M, PM, H = 8, 4, 512, 768
NH, HS = 12, 64
T = M * PM          # 2048 tokens per core
HC = H // 128       # 6 hidden chunks
TCM = PM // 128     # 4 token chunks per modality


def _emit(tc, ctx):
    nc = tc.nc

    x_ap = nc.dram_tensor("x", [H, T], F16, kind="ExternalInput").ap()
    wq_ap = nc.dram_tensor("wq", [H, H], F16, kind="ExternalInput").ap()
    wk_ap = nc.dram_tensor("wk", [H, H], F16, kind="ExternalInput").ap()
    wv_ap = nc.dram_tensor("wv", [H, H], F16, kind="ExternalInput").ap()
    wo_ap = nc.dram_tensor("wo", [H, H], F16, kind="ExternalInput").ap()
    out_ap = nc.dram_tensor("out", [T, H], F16, kind="ExternalOutput").ap()
    srf_ap = nc.dram_tensor("srf", [M * NH, 512], F32, kind="Internal").ap()

    const = ctx.enter_context(tc.tile_pool(name="const", bufs=1))

    # f32r tiles can't be written by memset/affine_select directly (no
    # f32r rounding on those ISA paths); stage in f32 and copy via DVE.
    onescol = const.tile([128, NH * TCM], F16)
    onesrow = const.tile([1, 128], F16)
    warm = const.tile([128, 512], F16)
    nc.gpsimd.memset(warm[:], 0.0)
    expwarm = const.tile([1, 16], F32)
    with tc.tile_pool(name="stage", bufs=1) as stage:
        ones_stage = stage.tile([128, 128], F32)
        nc.gpsimd.memset(ones_stage[:], 1.0)
        nc.vector.tensor_copy(onescol[:], ones_stage[:, :NH * TCM])
        nc.vector.tensor_copy(onesrow[:], ones_stage[0:1, :])

    wpool = ctx.enter_context(tc.tile_pool(name="w", bufs=1))
    xtp = ctx.enter_context(tc.tile_pool(name="xt", bufs=2))
    qpool = ctx.enter_context(tc.tile_pool(name="q", bufs=2))
    kpool = ctx.enter_context(tc.tile_pool(name="k", bufs=2))
    vpool = ctx.enter_context(tc.tile_pool(name="v", bufs=2))
    epool = ctx.enter_context(tc.tile_pool(name="e", bufs=4))
    stpool = ctx.enter_context(tc.tile_pool(name="st", bufs=4))
    bcpool = ctx.enter_context(tc.tile_pool(name="bc", bufs=5))
    cpool = ctx.enter_context(tc.tile_pool(name="ctx", bufs=1))
    opool = ctx.enter_context(tc.tile_pool(name="o", bufs=4))
    ps_big = ctx.enter_context(tc.tile_pool(name="ps_big", bufs=2, space="PSUM"))
    ps_sc = ctx.enter_context(tc.tile_pool(name="ps_sc", bufs=2, space="PSUM"))
    ps_c = ctx.enter_context(tc.tile_pool(name="ps_c", bufs=2, space="PSUM"))

    w_tiles = {}

    def load_weights():
        # wk behind x0 on the scalar queue; wv/wo behind wq on sync, so the
        # k-projection bootstrap isn't starved waiting for wv/wo bytes.
        for name, ap, eng in (("wk", wk_ap, nc.sync), ("wv", wv_ap, nc.scalar),
                              ("wo", wo_ap, nc.scalar)):
            t = wpool.tile([128, HC, H], F16, tag=name)
            src = ap.rearrange("(kc p) j -> p kc j", p=128)
            for kc in range(HC):
                eng.dma_start(t[:, kc, :], src[:, kc, :])
            w_tiles[name] = t

    mod = {}

    def emit_load_x(m):
        xt = xtp.tile([128, HC, PM], F16, tag="xt")
        if m == 0:
            # Interleave x and Wq chunk DMAs on the two HW DGE queues so the
            # first projection group's operands land as early as possible.
            wq = wpool.tile([128, HC, H], F16, tag="wq", name="wq")
            w_tiles["wq"] = wq
            wq_src = wq_ap.rearrange("(kc p) j -> p kc j", p=128)
            for hc in range(HC):
                nc.scalar.dma_start(
                    xt[:, hc, :],
                    x_ap.rearrange("(hc p) t -> p hc t", p=128)[:, hc, :PM],
                )
                nc.sync.dma_start(wq[:, hc, :], wq_src[:, hc, :])
            mod[m] = {"xt": xt}
            load_weights()
            return
        for hc in range(HC):
            nc.scalar.dma_start(
                xt[:, hc, :],
                x_ap.rearrange("(hc p) t -> p hc t", p=128)[:, hc, m * PM:(m + 1) * PM],
            )
        mod[m] = {"xt": xt}

    def evac_k(kt, jc, ps, eng_idx):
        # kt is the zero-padded per-head slab layout [128, NH, PM]: head 2jc
        # lives in rows 0-63 of slab 2jc, head 2jc+1 in rows 64-127 of slab
        # 2jc+1; the complementary halves stay zero (boot memset) so scores
        # can contract over all 128 partitions.
        for half, hh in ((0, 2 * jc), (64, 2 * jc + 1)):
            src = ps[half:half + 64, :]
            dst = kt[half:half + 64, hh, :]
            if eng_idx % 2 == 0:
                nc.vector.tensor_copy(dst, src)
            else:
                nc.scalar.activation(dst, src, AF.Copy)

    def proj_qk_group(m, which, jc):
        st = mod[m]
        key = "qt" if which == "q" else "kt"
        if key not in st:
            if which == "q":
                st[key] = qpool.tile([128, HC, PM], F16, tag="q", name="qt")
            else:
                st[key] = kpool.tile([128, NH, PM], F16, tag="k", name="kt")
        w = w_tiles["wq" if which == "q" else "wk"]
        ps = ps_big.tile([128, 512], F32, tag="ps_big")
        for kc in range(HC):
            nc.tensor.matmul(
                ps[:],
                w[:, kc, jc * 128:(jc + 1) * 128],
                st["xt"][:, kc, :],
                start=(kc == 0),
                stop=(kc == HC - 1),
            )
        if which == "k":
            evac_k(st[key], jc, ps, jc)
        elif jc % 2 == 0:
            nc.vector.tensor_copy(st[key][:, jc, :], ps[:])
        else:
            nc.scalar.activation(st[key][:, jc, :], ps[:], AF.Copy)

    def proj_v_group(m, ti, nn):
        # vt is flat [128, TCM, NH*(HS+1)+63]: head h's (v | ones) slab at
        # offset h*65; the +63 tail lets the PV stationary read a full
        # 128-column slab even for the last head (overreads are finite
        # garbage that lands in unread PSUM rows 65..127).
        st = mod[m]
        if "vt" not in st:
            st["vt"] = vpool.tile([128, TCM, NH * (HS + 1) + 63], F16,
                                  tag="v", name="vt")
            nc.vector.tensor_copy(
                st["vt"][:, :, :NH * (HS + 1)]
                .rearrange("p t (h c) -> p t h c", c=HS + 1)[:, :, :, HS],
                onescol[:].rearrange("p (t h) -> p t h", t=TCM),
            )
        ps = ps_big.tile([128, 512], F32, tag="ps_big")
        for kc in range(HC):
            nc.tensor.matmul(
                ps[:, :384],
                st["xt"][:, kc, ti * 128:(ti + 1) * 128],
                w_tiles["wv"][:, kc, nn * 384:(nn + 1) * 384],
                start=(kc == 0),
                stop=(kc == HC - 1),
            )
        nc.scalar.activation(
            st["vt"][:, ti, nn * 390:(nn + 1) * 390]
            .rearrange("p (h c) -> p h c", c=HS + 1)[:, :, :HS],
            ps[:, :384].rearrange("p (h c) -> p h c", c=HS),
            AF.Copy,
        )

    def phase_ab_fillers(m):
        # qk groups are the per-head main fillers: they share the attention
        # matmuls' 128*128*512 config, so weaving them costs no PE reconfig.
        # The 384-wide v groups are spread as second fillers over the later
        # heads (a block at the boundary stalls on burst Scalar evacs).
        main = [lambda: emit_load_x(m)]
        for which in ("q", "k"):
            for jc in range(HC):
                main.append(lambda which=which, jc=jc: proj_qk_group(m, which, jc))
        vlist = [lambda ti=ti, nn=nn: proj_v_group(m, ti, nn)
                 for ti in range(TCM) for nn in range(2)]
        return main, vlist

    def drain_bcs(ctxt, bcs):
        for hh, bcv in bcs:
            hhc, hhr = hh // 2, (hh % 2) * 64
            nc.vector.tensor_tensor(
                ctxt[hhr:hhr + 64, hhc, :], ctxt[hhr:hhr + 64, hhc, :],
                bcv[hhr:hhr + 64, :], ALU.mult,
            )
        del bcs[:]

    def finish_head(m, item):
        # Fused evacuate+normalize: ctxt[h] = psc[:64] * (1/sums broadcast).
        # For the bounce path the broadcast arrived by DMA; for the PE path
        # broadcast 1/sums to all 128 partitions with a contract-1 matmul
        # (fills PE bubbles in the fillerless last modality).
        h, psc, bcv, rf16 = item
        hc, hr = h // 2, (h % 2) * 64
        if rf16 is not None:
            psb = ps_sc.tile([128, 512], F32, tag="ps_sc", name="psbc")
            nc.tensor.matmul(psb[:], onesrow[0:1, :], rf16[0:1, :],
                             start=True, stop=True)
            bcv = bcpool.tile([128, 512], F32, tag="bc", name="bcsb")
            nc.vector.tensor_copy(bcv[hr:hr + 64, :], psb[hr:hr + 64, :])
        nc.vector.tensor_tensor(
            mod[m]["ctxt"][hr:hr + 64, hc, :], psc[:HS, :],
            bcv[hr:hr + 64, :], ALU.mult,
        )

    def attention(m, main, vlist, bcast=False):
        # Per (modality, head): scoresT on PE, exp on ScalarE, PV (with the
        # v_aug ones column producing softmax sums in psum row 64), then
        # 1/sums via reciprocal_approx_fast straight from PSUM and a fused
        # evacuate+normalize on DVE one head later.  Between each head's
        # scores and PV we weave one independent PE work unit (projections of
        # the next modality, or out-proj of the previous one) to fill the
        # exp wait.
        st = mod[m]
        qt, kt, vt = st["qt"], st["kt"], st["vt"]
        ctxt = cpool.tile([128, HC, PM], F16, tag="ctx")
        st["ctxt"] = ctxt
        pend = []
        bcs = []

        for h in range(NH):
            hc, hr = h // 2, (h % 2) * 64
            # kt slabs are zero-padded to 128 contract rows and vt slabs to
            # 128 stationary columns, so every attention matmul is the same
            # 128*128*512 config as the projections -- no PE reconfig
            # penalty between scores / PV / filler groups.
            qh = qt[:, hc, :]
            ets = []
            for jc2 in range(2):
                pst = ps_sc.tile([128, 2, 512], F32, tag="ps_sc",
                                 name=f"pst{jc2}")
                for j in range(2):
                    jc = jc2 * 2 + j
                    nc.tensor.matmul(
                        pst[:, j, :],
                        kt[:, h, jc * 128:(jc + 1) * 128],
                        qh,
                        start=True,
                        stop=True,
                    )
                et = epool.tile([128, 1024], F16, tag="e")
                nc.scalar.activation(et[:], pst[:], AF.Exp, scale=0.125)
                ets.append(et)
            if main:
                main.pop(0)()
                if h == 0 and len(main) > NH:
                    main.pop(0)()
            elif vlist:
                vlist.pop(0)()
            psc = ps_c.tile([128, 512], F32, tag="ps_c")
            for jc in range(TCM):
                nc.tensor.matmul(
                    psc[:],
                    vt[:, jc, h * (HS + 1):h * (HS + 1) + 128],
                    ets[jc // 2][:, (jc % 2) * 512:(jc % 2 + 1) * 512],
                    start=(jc == 0),
                    stop=(jc == TCM - 1),
                )
            stmp = stpool.tile([1, 512], F32, tag="stmp")
            nc.vector.tensor_copy(stmp[:], psc[HS:HS + 1, :])
            rf = stpool.tile([1, 512], F32, tag="rf")
            nc.vector.reciprocal_approx_fast(out=rf[:], in_=stmp[:])
            if not bcast:
                # Decoupled: plain evac releases the PSUM bank immediately;
                # the in-place normalize runs later, once the DMA-bounce
                # broadcast lands (batched so the PE never waits on it).
                nc.vector.tensor_copy(ctxt[hr:hr + 64, hc, :], psc[:HS, :])
                row = srf_ap[m * NH + h:m * NH + h + 1, :]
                nc.sync.dma_start(row, rf[0:1, :])
                bc = bcpool.tile([128, 512], F32, tag="bc")
                nc.sync.dma_start(bc[hr:hr + 64, :], row.to_broadcast((64, 512)))
                bcs.append((h, bc))
                if len(bcs) == 4 and h < NH - 1:
                    drain_bcs(ctxt, bcs)
            else:
                rf16 = stpool.tile([1, 512], F16, tag="rf16")
                nc.scalar.activation(rf16[:], rf[:], AF.Copy)
                pend.append((h, psc, None, rf16))
                while len(pend) > 1:
                    finish_head(m, pend.pop(0))
            if h >= NH - 8 and vlist:
                vlist.pop(0)()
        for f in main + vlist:
            f()
        del main[:], vlist[:]
        drain_bcs(ctxt, bcs)
        return pend

    def out_proj(m, pend=None, split=False):
        # For the final modality the last heads' normalize is still in
        # flight: emit the first two token tiles' cc0..4 partial sums to
        # cover it, then close with cc5 + evacuations.
        ctxt = mod[m]["ctxt"]

        def emit_mms(ti, nn, ps, ccs):
            for cc in ccs:
                nc.tensor.matmul(
                    ps[:, :384],
                    ctxt[:, cc, ti * 128:(ti + 1) * 128],
                    w_tiles["wo"][:, cc, nn * 384:(nn + 1) * 384],
                    start=(cc == 0),
                    stop=(cc == HC - 1),
                )

        def emit_tail(ti, nn, ps, osb):
            emit_mms(ti, nn, ps, range(5, HC))
            nc.scalar.activation(osb[:, nn * 384:(nn + 1) * 384],
                                 ps[:, :384], AF.Copy)

        if split:
            # cc0..3 only touches heads 0-7 (normalized long ago); emit all
            # four token tiles' partial sums first so the PE keeps running
            # while the last heads' normalize (bc bounce) lands, then close
            # each tile with cc4..5 + evacuation.
            psE = ps_sc.tile([128, 2, 512], F32, tag="ps_sc", name="psoE")
            psF = ps_sc.tile([128, 2, 512], F32, tag="ps_sc", name="psoF")
            psts = [
                ps_big.tile([128, 512], F32, tag="ps_big", name="psoA")[:],
                ps_big.tile([128, 512], F32, tag="ps_big", name="psoB")[:],
                psE[:, 0, :], psE[:, 1, :],
                psF[:, 0, :], psF[:, 1, :],
                ps_c.tile([128, 512], F32, tag="ps_c", name="psoG")[:],
                ps_c.tile([128, 512], F32, tag="ps_c", name="psoH")[:],
            ]
            for ti in range(TCM):
                for nn in range(2):
                    emit_mms(ti, nn, psts[ti * 2 + nn], range(4))
            while pend:
                finish_head(m, pend.pop(0))

            # Close cc4..5 one tile ahead of the evacuations, with the two
            # half-evacs split across Scalar and Vector, so the final drain
            # overlaps instead of serializing on one engine.
            osbs = [opool.tile([128, H], F16, tag="o", name=f"osbf{ti}")
                    for ti in range(TCM)]

            def close_mms(ti):
                for nn in range(2):
                    emit_mms(ti, nn, psts[ti * 2 + nn], range(4, HC))

            def close_evac(ti):
                nc.scalar.activation(osbs[ti][:, :384],
                                     psts[ti * 2][:, :384], AF.Copy)
                nc.vector.tensor_copy(osbs[ti][:, 384:],
                                      psts[ti * 2 + 1][:, :384])
                row0 = (m * TCM + ti) * 128
                nc.sync.dma_start(out_ap[row0:row0 + 128, :], osbs[ti][:])

            close_mms(0)
            close_mms(1)
            close_evac(0)
            close_mms(2)
            close_evac(1)
            close_mms(3)
            close_evac(2)
            close_evac(3)
            return
        for ti in range(TCM):
            osb = opool.tile([128, H], F16, tag="o")
            for nn in range(2):
                ps = ps_big.tile([128, 512], F32, tag="ps_big")
                emit_mms(ti, nn, ps, range(HC))
                nc.scalar.activation(osb[:, nn * 384:(nn + 1) * 384],
                                     ps[:, :384], AF.Copy)
            row0 = (m * TCM + ti) * 128
            nc.sync.dma_start(out_ap[row0:row0 + 128, :], osb[:])

    # PE warmup: dependency-free matmuls on a zeroed tile kick the DVFS
    # ramp while the first x/W DMA chunks are still in flight; also preload
    # the EXP activation table so the first real exp doesn't pay the
    # ~1.5us table load on the critical path.  The kt pad-halves are zeroed
    # once here (DVE is idle); the k evacs only ever write the live halves,
    # so the zeros survive the kpool buffer rotation across modalities.
    nc.scalar.activation(expwarm[:], warm[0:1, :16], AF.Exp, scale=0.125)
    for i in range(2):
        ktz = kpool.tile([128, NH, PM], F16, tag="k", name=f"ktz{i}")
        nc.vector.memset(ktz[0:64, 1::2, :], 0.0)
        nc.vector.memset(ktz[64:128, 0::2, :], 0.0)
    for _ in range(5):
        ps = ps_big.tile([128, 512], F32, tag="ps_big")
        nc.tensor.matmul(ps[:], warm[:, :128], warm[:], start=True, stop=True)

    # Modality 0 bootstrap: kc-OUTER accumulation across six PSUM banks, so
    # the first matmul only needs chunk 0 of wq/x and each kc step consumes
    # the next DMA chunk as it lands.
    emit_load_x(0)
    st0 = mod[0]
    st0["qt"] = qpool.tile([128, HC, PM], F16, tag="q", name="qt0")
    st0["kt"] = kpool.tile([128, NH, PM], F16, tag="k", name="kt0")
    for which in ("q", "k"):
        w = w_tiles["wq" if which == "q" else "wk"]
        psE = ps_sc.tile([128, 2, 512], F32, tag="ps_sc", name=f"bsE{which}")
        psF = ps_sc.tile([128, 2, 512], F32, tag="ps_sc", name=f"bsF{which}")
        banks = [
            ps_big.tile([128, 512], F32, tag="ps_big", name=f"bsA{which}")[:],
            ps_big.tile([128, 512], F32, tag="ps_big", name=f"bsB{which}")[:],
            psE[:, 0, :], psE[:, 1, :], psF[:, 0, :], psF[:, 1, :],
        ]
        for kc in range(HC):
            for jc in range(HC):
                nc.tensor.matmul(
                    banks[jc], w[:, kc, jc * 128:(jc + 1) * 128],
                    st0["xt"][:, kc, :], start=(kc == 0), stop=(kc == HC - 1))
        for jc in range(HC):
            if which == "k":
                evac_k(st0["kt"], jc, banks[jc], jc)
            elif jc % 2 == 0:
                nc.vector.tensor_copy(st0["qt"][:, jc, :], banks[jc])
            else:
                nc.scalar.activation(st0["qt"][:, jc, :], banks[jc], AF.Copy)
    for ti in range(TCM):
        for nn in range(2):
            proj_v_group(0, ti, nn)
    def op2_filler_units(m):
        # out_proj(m) sliced into per-(ti, nn) units, used as PE fillers
        # inside the (otherwise fillerless) last modality's attention.
        state = {}

        def unit(ti, nn):
            ctxt = mod[m]["ctxt"]
            if ti not in state:
                state[ti] = opool.tile([128, H], F16, tag="o", name=f"osb2_{ti}")
            osb = state[ti]
            ps = ps_big.tile([128, 512], F32, tag="ps_big")
            for cc in range(HC):
                nc.tensor.matmul(
                    ps[:, :384],
                    ctxt[:, cc, ti * 128:(ti + 1) * 128],
                    w_tiles["wo"][:, cc, nn * 384:(nn + 1) * 384],
                    start=(cc == 0),
                    stop=(cc == HC - 1),
                )
            nc.scalar.activation(osb[:, nn * 384:(nn + 1) * 384],
                                 ps[:, :384], AF.Copy)
            if nn == 1:
                row0 = (m * TCM + ti) * 128
                nc.sync.dma_start(out_ap[row0:row0 + 128, :], osb[:])

        return [lambda ti=ti, nn=nn: unit(ti, nn)
                for ti in range(TCM) for nn in range(2)]

    for m in range(M):
        if m + 1 < M:
            main, vlist = phase_ab_fillers(m + 1)
        else:
            main, vlist = op2_filler_units(M - 2), []
        pend = attention(m, main, vlist, bcast=False)
        if m == M - 1:
            out_proj(m, pend, split=True)
        else:
            while pend:
                finish_head(m, pend.pop(0))
            if m != M - 2:
                out_proj(m)

_NC_CACHE = {}


def build_nc():
    if "nc" not in _NC_CACHE:
        nc = bacc.Bacc("TRN2", target_bir_lowering=False, debug=False, num_devices=B)
        with TileContext(nc) as tc:
            with ExitStack() as stack:
                _emit(tc, stack)
        nc.compile()
        _NC_CACHE["nc"] = nc
    return _NC_CACHE["nc"]


def _numpy_fallback(x, Wq, bq, Wk, bk, Wv, bv, Wo, bo):
    Bb, Mm, Pp, Hh = x.shape
    xx = x.reshape(-1, Hh)
    q = (xx @ Wq + bq).reshape(Bb, Mm, Pp, NH, HS).transpose(0, 1, 3, 2, 4)
    k = (xx @ Wk + bk).reshape(Bb, Mm, Pp, NH, HS).transpose(0, 1, 3, 2, 4)
    v = (xx @ Wv + bv).reshape(Bb, Mm, Pp, NH, HS).transpose(0, 1, 3, 2, 4)
    s = np.einsum("bmnqh,bmnkh->bmnqk", q, k) / np.sqrt(HS)
    s = s - s.max(axis=-1, keepdims=True)
    e = np.exp(s)
    p = e / e.sum(axis=-1, keepdims=True)
    ctx = np.einsum("bmnqk,bmnkh->bmnqh", p, v)
    ctx = ctx.transpose(0, 1, 3, 2, 4).reshape(Bb, Mm, Pp, Hh)
    return (ctx @ Wo + bo).astype(np.float32)


def make_in_maps(hs, ws):
    """hs: [B,M,P,H] f32, ws: dict of f32 [H,H] -> per-core fp16 input maps."""
    ws16 = {n: np.ascontiguousarray(w.astype(np.float16)) for n, w in ws.items()}
    return [
        {"x": np.ascontiguousarray(hs[b].reshape(T, H).T.astype(np.float16)),
         **ws16}
        for b in range(B)
    ]


def kernel(hidden_states, Wq, bq, Wk, bk, Wv, bv, Wo, bo):
    hs = np.ascontiguousarray(np.asarray(hidden_states, dtype=np.float32))
    ws = {n: np.ascontiguousarray(np.asarray(w, dtype=np.float32))
          for n, w in (("wq", Wq), ("wk", Wk), ("wv", Wv), ("wo", Wo))}
    biases = [np.asarray(b, dtype=np.float32) for b in (bq, bk, bv, bo)]
    if any(np.any(b) for b in biases):
        return _numpy_fallback(hs, ws["wq"], biases[0], ws["wk"], biases[1],
                               ws["wv"], biases[2], ws["wo"], biases[3])

    in_maps = make_in_maps(hs, ws)
    # The device occasionally comes up wedged from a previous process
    # (NRT_EXEC_UNIT_UNRECOVERABLE); retry, then degrade to the (correct
    # but slow) numpy path rather than crash.
    last_exc = None
    for _ in range(3):
        try:
            nc = build_nc()
            res = bass_utils.run_bass_kernel_spmd(
                nc, in_maps, core_ids=list(range(B)))
            out = np.stack(
                [res.results[b]["out"].reshape(M, PM, H) for b in range(B)])
            return out.astype(np.float32)
        except Exception as e:  # noqa: BLE001
            last_exc = e
            import time
            time.sleep(2)
    import warnings
    warnings.warn(f"TRN execution failed ({last_exc!r}); numpy fallback")
    return _numpy_fallback(hs, ws["wq"], biases[0], ws["wk"], biases[1],
                           ws["wv"], biases[2], ws["wo"], biases[3])

